# revision 9
# baseline (speedup 1.0000x reference)
"""KAN-GNN message passing on 8 TRN2 NeuronCores.

Strategy (data-parallel over nodes, per sharding hint):
 - Nodes are ranked by in-degree and dealt round-robin to the 8 cores, so
   every core holds 12500 targets with a near-identical degree profile.
 - Per core: KAN layer 1 on its node shard (3 fused matmuls: x, x^2, x^3
   against host-combined weights, bias via PSUM pre-init, relu+pad-mask in
   the ACT drain), then an AllGather of the bf16 h1 table.
 - Aggregation = one indirect-DMA gather per 128-target group: each target
   (partition) pulls its padded list of source rows side by side in the
   free dim, then a contiguous halving tree of DVE adds reduces the slots;
   scale by 1/deg, PE-transpose to put features on partitions, KAN layer 2,
   AllGather of h2, second gather/reduce, log_softmax.
 - All indices/permutations are precomputed on the host and baked into the
   (single, SPMD) program; per-core data goes in as input tensors.
"""
import numpy as np
import ml_dtypes

import concourse.bacc as bacc
import concourse.mybir as mybir
import concourse.tile as tile
import concourse.bass as bass
from concourse.bass_utils import run_bass_kernel_spmd

N_NODES = 100000
N_EDGES = 1600000
IN_F, HID_F, OUT_F = 128, 128, 64
K = 8               # cores
P = 128             # partitions / targets per group
J = 12544           # local nodes per core (98 * 128), 12500 real + 44 pad
G = J // P          # 98 groups
JREAL = N_NODES // K  # 12500
TBL = K * J         # 100352 rows in the all-gathered tables
PAD_POS = JREAL     # position (core 0, j=12500) -> guaranteed zero row

BF16 = mybir.dt.bfloat16
F32 = mybir.dt.float32
I32 = mybir.dt.int32


def _host_prep(x, edge_index, w1, b1, c1, w2, b2, c2):
    src = np.asarray(edge_index[0], dtype=np.int64)
    tgt = np.asarray(edge_index[1], dtype=np.int64)
    x = np.asarray(x, dtype=np.float32)

    deg = np.bincount(tgt, minlength=N_NODES)
    order = np.argsort(-deg, kind="stable")          # global degree rank -> node
    rank_of = np.empty(N_NODES, dtype=np.int64)
    rank_of[order] = np.arange(N_NODES)
    core_of_node = rank_of % K
    j_of_node = rank_of // K
    pos_of_node = core_of_node * J + j_of_node        # row in AG tables

    # per-core local degree [K, J]
    degs_kj = np.zeros((K, J), dtype=np.int64)
    degs_kj[core_of_node, j_of_node] = deg

    # group slot counts (shared across cores -> same program)
    Dg = degs_kj.reshape(K, G, P).max(axis=(0, 2))    # [G]
    Dg = np.maximum(Dg, 1).astype(np.int64)
    offs = np.concatenate([[0], np.cumsum(Dg)])       # [G+1]
    S = int(offs[-1])

    # slot table: idx_all[k, p, col] = table position of the d-th source of
    # local target (g*128+p) on core k; PAD_POS when d >= degree.
    idx_all = np.full((K, P, S), PAD_POS, dtype=np.int32)
    ek = core_of_node[tgt]
    ej = j_of_node[tgt]
    key = ek * J + ej
    eorder = np.argsort(key, kind="stable")
    skey = key[eorder]
    ukey, counts = np.unique(skey, return_counts=True)
    run_starts = np.concatenate([[0], np.cumsum(counts)[:-1]])
    d_in_run = np.arange(len(skey)) - np.repeat(run_starts, counts)
    ks = skey // J
    js = skey % J
    gs = js // P
    ps = js % P
    cols = offs[gs] + d_in_run
    idx_all[ks, ps, cols] = pos_of_node[src[eorder]].astype(np.int32)

    # per-core 1/deg  [K, P, G]  (0 for pad targets)
    with np.errstate(divide="ignore"):
        dr = 1.0 / np.maximum(degs_kj, 1).astype(np.float32)
    real = np.zeros((K, J), dtype=np.float32)
    real[:, :JREAL] = 1.0
    # reference divides by max(deg,1); deg-0 real targets get sum 0 -> 0 fine
    degrecip = (dr * np.where(real > 0, 1.0, 0.0)).reshape(K, G, P).transpose(0, 2, 1).copy()

    # pad-node mask [P, G] (same on every core)
    mask_j = (np.arange(J) < JREAL).astype(np.float32)
    mask_pg = mask_j.reshape(G, P).T.copy()

    # xT shards, bf16 [K][IN_F, J]
    xT = np.zeros((K, IN_F, J), dtype=ml_dtypes.bfloat16)
    for k in range(K):
        nodes_k = order[np.arange(JREAL) * K + k]
        xT[k, :, :JREAL] = x[nodes_k].T.astype(ml_dtypes.bfloat16)

    # fused KAN weights
    A1 = (w1 + 0.1 * c1[:, :, 0]).astype(ml_dtypes.bfloat16)
    B1 = (0.1 * c1[:, :, 1]).astype(ml_dtypes.bfloat16)
    C1 = (0.1 * c1[:, :, 2]).astype(ml_dtypes.bfloat16)
    A2 = (w2 + 0.1 * c2[:, :, 0]).astype(ml_dtypes.bfloat16)
    B2 = (0.1 * c2[:, :, 1]).astype(ml_dtypes.bfloat16)
    C2 = (0.1 * c2[:, :, 2]).astype(ml_dtypes.bfloat16)
    b1b = np.tile(np.asarray(b1, np.float32)[None, :], (P, 1))
    b2b = np.tile(np.asarray(b2, np.float32)[None, :], (P, 1))
    ident = np.eye(P, dtype=np.float32)

    in_maps = []
    for k in range(K):
        in_maps.append({
            "xT": xT[k],
            "idx": idx_all[k],
            "degrecip": degrecip[k],
            "mask": mask_pg,
            "A1": A1, "B1": B1, "C1": C1,
            "A2": A2, "B2": B2, "C2": C2,
            "b1b": b1b, "b2b": b2b, "ident": ident,
        })
    meta = {"Dg": Dg, "offs": offs, "S": S, "order": order}
    return in_maps, meta


def _tree_reduce(nc, tiles_ap, D, F):
    """In-place halving tree over D slots of width F. Returns slice [P, F]."""
    cur = D
    while cur > 1:
        h = cur // 2
        nc.vector.tensor_tensor(
            out=tiles_ap[:, : h * F],
            in0=tiles_ap[:, : h * F],
            in1=tiles_ap[:, (cur - h) * F: cur * F],
            op=mybir.AluOpType.add,
        )
        cur = cur - h
    return tiles_ap[:, :F]


def build_program(Dg, offs, S, dump=False):
    nc = bacc.Bacc("TRN2", target_bir_lowering=False, debug=False, num_devices=K)

    xT = nc.dram_tensor("xT", [IN_F, J], BF16, kind="ExternalInput")
    idx = nc.dram_tensor("idx", [P, S], I32, kind="ExternalInput")
    degrecip = nc.dram_tensor("degrecip", [P, G], F32, kind="ExternalInput")
    mask = nc.dram_tensor("mask", [P, G], F32, kind="ExternalInput")
    A1 = nc.dram_tensor("A1", [IN_F, HID_F], BF16, kind="ExternalInput")
    B1 = nc.dram_tensor("B1", [IN_F, HID_F], BF16, kind="ExternalInput")
    C1 = nc.dram_tensor("C1", [IN_F, HID_F], BF16, kind="ExternalInput")
    A2 = nc.dram_tensor("A2", [HID_F, OUT_F], BF16, kind="ExternalInput")
    B2 = nc.dram_tensor("B2", [HID_F, OUT_F], BF16, kind="ExternalInput")
    C2 = nc.dram_tensor("C2", [HID_F, OUT_F], BF16, kind="ExternalInput")
    b1b = nc.dram_tensor("b1b", [P, HID_F], F32, kind="ExternalInput")
    b2b = nc.dram_tensor("b2b", [P, OUT_F], F32, kind="ExternalInput")
    ident = nc.dram_tensor("ident", [P, P], F32, kind="ExternalInput")
    y = nc.dram_tensor("y", [J, OUT_F], F32, kind="ExternalOutput")
    if dump:
        h1o = nc.dram_tensor("h1o", [J, HID_F], BF16, kind="ExternalOutput")
        sno = nc.dram_tensor("sno", [J, HID_F], F32, kind="ExternalOutput")

    h1_in = nc.dram_tensor("h1_in", [J, HID_F], BF16, kind="Internal")
    h1_tbl = nc.dram_tensor("h1_tbl", [TBL, HID_F], BF16, kind="Internal",
                            addr_space="Shared")
    h2_in = nc.dram_tensor("h2_in", [J, OUT_F], BF16, kind="Internal")
    h2_tbl = nc.dram_tensor("h2_tbl", [TBL, OUT_F], BF16, kind="Internal",
                            addr_space="Shared")

    Dmax = int(max(Dg))

    with tile.TileContext(nc) as tc:
        with (
            tc.tile_pool(name="consts", bufs=1) as cpool,
            tc.tile_pool(name="work", bufs=3) as wpool,
            tc.tile_pool(name="gather", bufs=3) as gpool,
            tc.tile_pool(name="psum", bufs=2, space="PSUM") as ppool,
        ):
            # load constants
            c_idx = cpool.tile([P, S], I32, tag="idx")
            nc.sync.dma_start(out=c_idx[:], in_=idx[:, :])
            c_dr = cpool.tile([P, G], F32, tag="dr")
            nc.sync.dma_start(out=c_dr[:], in_=degrecip[:, :])
            c_mask = cpool.tile([P, G], F32, tag="mask")
            nc.sync.dma_start(out=c_mask[:], in_=mask[:, :])
            c_w1 = []
            for nm, t in (("A1", A1), ("B1", B1), ("C1", C1)):
                w = cpool.tile([IN_F, HID_F], BF16, tag=nm)
                nc.sync.dma_start(out=w[:], in_=t[:, :])
                c_w1.append(w)
            c_w2 = []
            for nm, t in (("A2", A2), ("B2", B2), ("C2", C2)):
                w = cpool.tile([HID_F, OUT_F], BF16, tag=nm)
                nc.sync.dma_start(out=w[:], in_=t[:, :])
                c_w2.append(w)
            c_b1 = cpool.tile([P, HID_F], F32, tag="b1b")
            nc.sync.dma_start(out=c_b1[:], in_=b1b[:, :])
            c_b2 = cpool.tile([P, OUT_F], F32, tag="b2b")
            nc.sync.dma_start(out=c_b2[:], in_=b2b[:, :])
            c_id = cpool.tile([P, P], F32, tag="ident")
            nc.sync.dma_start(out=c_id[:], in_=ident[:, :])

            # ---------------- phase 1: KAN layer 1 on the shard ----------------
            for g in range(G):
                xt = wpool.tile([IN_F, P], BF16, tag="xt")
                nc.sync.dma_start(out=xt[:], in_=xT[:, g * P:(g + 1) * P])
                x2 = wpool.tile([IN_F, P], BF16, tag="x2")
                nc.vector.tensor_tensor(out=x2[:], in0=xt[:], in1=xt[:],
                                        op=mybir.AluOpType.mult)
                x3 = wpool.tile([IN_F, P], BF16, tag="x3")
                nc.vector.tensor_tensor(out=x3[:], in0=x2[:], in1=xt[:],
                                        op=mybir.AluOpType.mult)
                ps = ppool.tile([P, HID_F], F32, tag="k1")
                nc.tensor.matmul(out=ps[:], lhsT=xt[:], rhs=c_w1[0][:],
                                 start=True, stop=False)
                nc.tensor.matmul(out=ps[:], lhsT=x2[:], rhs=c_w1[1][:],
                                 start=False, stop=False)
                nc.tensor.matmul(out=ps[:], lhsT=x3[:], rhs=c_w1[2][:],
                                 start=False, stop=True)
                hb = wpool.tile([P, HID_F], F32, tag="hb")
                nc.vector.tensor_tensor(out=hb[:], in0=ps[:], in1=c_b1[:],
                                        op=mybir.AluOpType.add)
                h1t = wpool.tile([P, HID_F], BF16, tag="h1t")
                nc.scalar.activation(out=h1t[:], in_=hb[:],
                                     func=mybir.ActivationFunctionType.Relu,
                                     scale=c_mask[:, g:g + 1])
                nc.sync.dma_start(out=h1_in[g * P:(g + 1) * P, :], in_=h1t[:])
                if dump:
                    nc.sync.dma_start(out=h1o[g * P:(g + 1) * P, :], in_=h1t[:])

            # ---------------- AllGather h1 ----------------
            nc.gpsimd.collective_compute(
                "AllGather", mybir.AluOpType.bypass,
                replica_groups=[list(range(K))],
                ins=[h1_in[:, :]], outs=[h1_tbl[:, :]],
            )

            # ---------------- phase 2: aggregate + KAN layer 2 ----------------
            for g in range(G):
                D = int(Dg[g])
                off = int(offs[g])
                gt = gpool.tile([P, Dmax * HID_F], BF16, tag="g1")
                for d in range(D):
                    nc.gpsimd.indirect_dma_start(
                        out=gt[:, d * HID_F:(d + 1) * HID_F],
                        out_offset=None,
                        in_=h1_tbl[:, :],
                        in_offset=bass.IndirectOffsetOnAxis(
                            ap=c_idx[:, off + d:off + d + 1], axis=0),
                    )
                s1 = _tree_reduce(nc, gt, D, HID_F)
                sn = wpool.tile([P, HID_F], F32, tag="sn")
                nc.vector.tensor_scalar_mul(sn[:], s1, c_dr[:, g:g + 1])
                if dump:
                    nc.sync.dma_start(out=sno[g * P:(g + 1) * P, :], in_=sn[:])
                pt = ppool.tile([P, P], F32, tag="tr")
                nc.tensor.transpose(out=pt[:], in_=sn[:], identity=c_id[:])
                hT = wpool.tile([HID_F, P], BF16, tag="hT")
                nc.scalar.copy(out=hT[:], in_=pt[:])
                q2 = wpool.tile([HID_F, P], BF16, tag="q2")
                nc.vector.tensor_tensor(out=q2[:], in0=hT[:], in1=hT[:],
                                        op=mybir.AluOpType.mult)
                q3 = wpool.tile([HID_F, P], BF16, tag="q3")
                nc.vector.tensor_tensor(out=q3[:], in0=q2[:], in1=hT[:],
                                        op=mybir.AluOpType.mult)
                ps2 = ppool.tile([P, OUT_F], F32, tag="k2")
                nc.tensor.matmul(out=ps2[:], lhsT=hT[:], rhs=c_w2[0][:],
                                 start=True, stop=False)
                nc.tensor.matmul(out=ps2[:], lhsT=q2[:], rhs=c_w2[1][:],
                                 start=False, stop=False)
                nc.tensor.matmul(out=ps2[:], lhsT=q3[:], rhs=c_w2[2][:],
                                 start=False, stop=True)
                hb2 = wpool.tile([P, OUT_F], F32, tag="hb2")
                nc.vector.tensor_tensor(out=hb2[:], in0=ps2[:], in1=c_b2[:],
                                        op=mybir.AluOpType.add)
                h2t = wpool.tile([P, OUT_F], BF16, tag="h2t")
                nc.scalar.activation(out=h2t[:], in_=hb2[:],
                                     func=mybir.ActivationFunctionType.Copy,
                                     scale=c_mask[:, g:g + 1])
                nc.sync.dma_start(out=h2_in[g * P:(g + 1) * P, :], in_=h2t[:])

            # ---------------- AllGather h2 ----------------
            nc.gpsimd.collective_compute(
                "AllGather", mybir.AluOpType.bypass,
                replica_groups=[list(range(K))],
                ins=[h2_in[:, :]], outs=[h2_tbl[:, :]],
            )

            # ---------------- phase 3: aggregate + log_softmax ----------------
            for g in range(G):
                D = int(Dg[g])
                off = int(offs[g])
                gt = gpool.tile([P, Dmax * OUT_F], BF16, tag="g2")
                for d in range(D):
                    nc.gpsimd.indirect_dma_start(
                        out=gt[:, d * OUT_F:(d + 1) * OUT_F],
                        out_offset=None,
                        in_=h2_tbl[:, :],
                        in_offset=bass.IndirectOffsetOnAxis(
                            ap=c_idx[:, off + d:off + d + 1], axis=0),
                    )
                s2 = _tree_reduce(nc, gt, D, OUT_F)
                tn = wpool.tile([P, OUT_F], F32, tag="tn")
                nc.vector.tensor_scalar_mul(tn[:], s2, c_dr[:, g:g + 1])
                mx = wpool.tile([P, 1], F32, tag="mx")
                nc.vector.tensor_reduce(out=mx[:], in_=tn[:],
                                        axis=mybir.AxisListType.X,
                                        op=mybir.AluOpType.max)
                nmx = wpool.tile([P, 1], F32, tag="nmx")
                nc.vector.tensor_scalar_mul(nmx[:], mx[:], -1.0)
                et = wpool.tile([P, OUT_F], F32, tag="et")
                se = wpool.tile([P, 1], F32, tag="se")
                nc.scalar.activation(out=et[:], in_=tn[:],
                                     func=mybir.ActivationFunctionType.Exp,
                                     bias=nmx[:, :1], scale=1.0,
                                     accum_out=se[:, :1])
                lse = wpool.tile([P, 1], F32, tag="lse")
                nc.scalar.activation(out=lse[:], in_=se[:],
                                     func=mybir.ActivationFunctionType.Ln)
                ot = wpool.tile([P, OUT_F], F32, tag="ot")
                nc.vector.tensor_scalar(ot[:], tn[:], nmx[:, :1], lse[:, :1],
                                        mybir.AluOpType.add,
                                        mybir.AluOpType.subtract)
                nc.sync.dma_start(out=y[g * P:(g + 1) * P, :], in_=ot[:])

    nc.compile()
    return nc


def kernel(x, edge_index, w1, b1, c1, w2, b2, c2):
    in_maps, meta = _host_prep(x, edge_index, w1, b1, c1, w2, b2, c2)
    nc = build_program(meta["Dg"], meta["offs"], meta["S"])
    res = run_bass_kernel_spmd(nc, in_maps, core_ids=list(range(K)))
    order = meta["order"]
    out = np.empty((N_NODES, OUT_F), dtype=np.float32)
    jr = np.arange(JREAL)
    for k in range(K):
        out[order[jr * K + k]] = res.results[k]["y"][:JREAL]
    return out


# revision 10
# speedup vs baseline: 36.1480x; 36.1480x over previous
"""KAN-GNN message passing on 8 TRN2 NeuronCores.

Strategy (data-parallel over nodes, per sharding hint):
 - Nodes are ranked by in-degree and dealt round-robin to the 8 cores, so
   every core holds 12500 targets with a near-identical degree profile.
 - Per core: KAN layer 1 on its node shard (3 fused matmuls: x, x^2, x^3
   against host-combined weights, bias via PSUM pre-init, relu+pad-mask in
   the ACT drain), then an AllGather of the bf16 h1 table.
 - Aggregation = one indirect-DMA gather per 128-target group: each target
   (partition) pulls its padded list of source rows side by side in the
   free dim, then a contiguous halving tree of DVE adds reduces the slots;
   scale by 1/deg, PE-transpose to put features on partitions, KAN layer 2,
   AllGather of h2, second gather/reduce, log_softmax.
 - All indices/permutations are precomputed on the host and baked into the
   (single, SPMD) program; per-core data goes in as input tensors.
"""
import numpy as np
import ml_dtypes

import concourse.bacc as bacc
import concourse.mybir as mybir
import concourse.tile as tile
import concourse.bass as bass
from concourse.bass_utils import run_bass_kernel_spmd

N_NODES = 100000
N_EDGES = 1600000
IN_F, HID_F, OUT_F = 128, 128, 64
K = 8               # cores
P = 128             # partitions / targets per group
J = 12544           # local nodes per core (98 * 128), 12500 real + 44 pad
G = J // P          # 98 groups
JREAL = N_NODES // K  # 12500
TBL = K * J         # 100352 rows in the all-gathered tables
PAD_POS = JREAL     # position (core 0, j=12500) -> guaranteed zero row

BF16 = mybir.dt.bfloat16
F32 = mybir.dt.float32
I32 = mybir.dt.int32


def _host_prep(x, edge_index, w1, b1, c1, w2, b2, c2):
    src = np.asarray(edge_index[0], dtype=np.int64)
    tgt = np.asarray(edge_index[1], dtype=np.int64)
    x = np.asarray(x, dtype=np.float32)

    deg = np.bincount(tgt, minlength=N_NODES)
    order = np.argsort(-deg, kind="stable")          # global degree rank -> node
    rank_of = np.empty(N_NODES, dtype=np.int64)
    rank_of[order] = np.arange(N_NODES)
    core_of_node = rank_of % K
    j_of_node = rank_of // K
    pos_of_node = core_of_node * J + j_of_node        # row in AG tables

    # per-core local degree [K, J]
    degs_kj = np.zeros((K, J), dtype=np.int64)
    degs_kj[core_of_node, j_of_node] = deg

    # group slot counts (shared across cores -> same program)
    Dg = degs_kj.reshape(K, G, P).max(axis=(0, 2))    # [G]
    Dg = np.maximum(Dg, 1).astype(np.int64)
    offs = np.concatenate([[0], np.cumsum(Dg)])       # [G+1]
    S = int(offs[-1])

    # slot table: idx_all[k, p, col] = table position of the d-th source of
    # local target (g*128+p) on core k; PAD_POS when d >= degree.
    idx_all = np.full((K, P, S), PAD_POS, dtype=np.int32)
    ek = core_of_node[tgt]
    ej = j_of_node[tgt]
    key = ek * J + ej
    eorder = np.argsort(key, kind="stable")
    skey = key[eorder]
    ukey, counts = np.unique(skey, return_counts=True)
    run_starts = np.concatenate([[0], np.cumsum(counts)[:-1]])
    d_in_run = np.arange(len(skey)) - np.repeat(run_starts, counts)
    ks = skey // J
    js = skey % J
    gs = js // P
    ps = js % P
    cols = offs[gs] + d_in_run
    idx_all[ks, ps, cols] = pos_of_node[src[eorder]].astype(np.int32)

    # per-core 1/deg  [K, P, G]  (0 for pad targets)
    with np.errstate(divide="ignore"):
        dr = 1.0 / np.maximum(degs_kj, 1).astype(np.float32)
    real = np.zeros((K, J), dtype=np.float32)
    real[:, :JREAL] = 1.0
    # reference divides by max(deg,1); deg-0 real targets get sum 0 -> 0 fine
    degrecip = (dr * np.where(real > 0, 1.0, 0.0)).reshape(K, G, P).transpose(0, 2, 1).copy()

    # pad-node mask [P, G] (same on every core)
    mask_j = (np.arange(J) < JREAL).astype(np.float32)
    mask_pg = mask_j.reshape(G, P).T.copy()

    # xT shards, bf16 [K][IN_F, J]
    xT = np.zeros((K, IN_F, J), dtype=ml_dtypes.bfloat16)
    for k in range(K):
        nodes_k = order[np.arange(JREAL) * K + k]
        xT[k, :, :JREAL] = x[nodes_k].T.astype(ml_dtypes.bfloat16)

    # fused KAN weights
    A1 = (w1 + 0.1 * c1[:, :, 0]).astype(ml_dtypes.bfloat16)
    B1 = (0.1 * c1[:, :, 1]).astype(ml_dtypes.bfloat16)
    C1 = (0.1 * c1[:, :, 2]).astype(ml_dtypes.bfloat16)
    A2 = (w2 + 0.1 * c2[:, :, 0]).astype(ml_dtypes.bfloat16)
    B2 = (0.1 * c2[:, :, 1]).astype(ml_dtypes.bfloat16)
    C2 = (0.1 * c2[:, :, 2]).astype(ml_dtypes.bfloat16)
    b1b = np.tile(np.asarray(b1, np.float32)[None, :], (P, 1))
    b2b = np.tile(np.asarray(b2, np.float32)[None, :], (P, 1))
    ident = np.eye(P, dtype=np.float32)

    in_maps = []
    for k in range(K):
        in_maps.append({
            "xT": xT[k],
            "idx": idx_all[k],
            "degrecip": degrecip[k],
            "mask": mask_pg,
            "A1": A1, "B1": B1, "C1": C1,
            "A2": A2, "B2": B2, "C2": C2,
            "b1b": b1b, "b2b": b2b, "ident": ident,
        })
    meta = {"Dg": Dg, "offs": offs, "S": S, "order": order}
    return in_maps, meta


def _tree_reduce(nc, tiles_ap, D, F):
    """In-place halving tree over D slots of width F. Returns slice [P, F]."""
    cur = D
    while cur > 1:
        h = cur // 2
        nc.vector.tensor_tensor(
            out=tiles_ap[:, : h * F],
            in0=tiles_ap[:, : h * F],
            in1=tiles_ap[:, (cur - h) * F: cur * F],
            op=mybir.AluOpType.add,
        )
        cur = cur - h
    return tiles_ap[:, :F]


def build_program(Dg, offs, S, dump=False):
    nc = bacc.Bacc("TRN2", target_bir_lowering=False, debug=False, num_devices=K,
                   dynamic_dma_scratch_size=131072)

    xT = nc.dram_tensor("xT", [IN_F, J], BF16, kind="ExternalInput")
    idx = nc.dram_tensor("idx", [P, S], I32, kind="ExternalInput")
    degrecip = nc.dram_tensor("degrecip", [P, G], F32, kind="ExternalInput")
    mask = nc.dram_tensor("mask", [P, G], F32, kind="ExternalInput")
    A1 = nc.dram_tensor("A1", [IN_F, HID_F], BF16, kind="ExternalInput")
    B1 = nc.dram_tensor("B1", [IN_F, HID_F], BF16, kind="ExternalInput")
    C1 = nc.dram_tensor("C1", [IN_F, HID_F], BF16, kind="ExternalInput")
    A2 = nc.dram_tensor("A2", [HID_F, OUT_F], BF16, kind="ExternalInput")
    B2 = nc.dram_tensor("B2", [HID_F, OUT_F], BF16, kind="ExternalInput")
    C2 = nc.dram_tensor("C2", [HID_F, OUT_F], BF16, kind="ExternalInput")
    b1b = nc.dram_tensor("b1b", [P, HID_F], F32, kind="ExternalInput")
    b2b = nc.dram_tensor("b2b", [P, OUT_F], F32, kind="ExternalInput")
    ident = nc.dram_tensor("ident", [P, P], F32, kind="ExternalInput")
    y = nc.dram_tensor("y", [J, OUT_F], F32, kind="ExternalOutput")
    if dump:
        h1o = nc.dram_tensor("h1o", [J, HID_F], BF16, kind="ExternalOutput")
        sno = nc.dram_tensor("sno", [J, HID_F], F32, kind="ExternalOutput")

    h1_in = nc.dram_tensor("h1_in", [J, HID_F], BF16, kind="Internal")
    h1_tbl = nc.dram_tensor("h1_tbl", [TBL, HID_F], BF16, kind="Internal",
                            addr_space="Shared")
    h2_in = nc.dram_tensor("h2_in", [J, OUT_F], BF16, kind="Internal")
    h2_tbl = nc.dram_tensor("h2_tbl", [TBL, OUT_F], BF16, kind="Internal",
                            addr_space="Shared")

    Dmax = int(max(Dg))

    with tile.TileContext(nc) as tc:
        with (
            tc.tile_pool(name="consts", bufs=1) as cpool,
            tc.tile_pool(name="work", bufs=3) as wpool,
            tc.tile_pool(name="gather", bufs=3) as gpool,
            tc.tile_pool(name="psum", bufs=2, space="PSUM") as ppool,
        ):
            # load constants
            c_idx = cpool.tile([P, S], I32, tag="idx")
            nc.sync.dma_start(out=c_idx[:], in_=idx[:, :])
            c_dr = cpool.tile([P, G], F32, tag="dr")
            nc.sync.dma_start(out=c_dr[:], in_=degrecip[:, :])
            c_mask = cpool.tile([P, G], F32, tag="mask")
            nc.sync.dma_start(out=c_mask[:], in_=mask[:, :])
            c_w1 = []
            for nm, t in (("A1", A1), ("B1", B1), ("C1", C1)):
                w = cpool.tile([IN_F, HID_F], BF16, tag=nm)
                nc.sync.dma_start(out=w[:], in_=t[:, :])
                c_w1.append(w)
            c_w2 = []
            for nm, t in (("A2", A2), ("B2", B2), ("C2", C2)):
                w = cpool.tile([HID_F, OUT_F], BF16, tag=nm)
                nc.sync.dma_start(out=w[:], in_=t[:, :])
                c_w2.append(w)
            c_b1 = cpool.tile([P, HID_F], F32, tag="b1b")
            nc.sync.dma_start(out=c_b1[:], in_=b1b[:, :])
            c_b2 = cpool.tile([P, OUT_F], F32, tag="b2b")
            nc.sync.dma_start(out=c_b2[:], in_=b2b[:, :])
            c_id = cpool.tile([P, P], F32, tag="ident")
            nc.sync.dma_start(out=c_id[:], in_=ident[:, :])

            # ---------------- phase 1: KAN layer 1 on the shard ----------------
            for g in range(G):
                xt = wpool.tile([IN_F, P], BF16, tag="xt")
                nc.sync.dma_start(out=xt[:], in_=xT[:, g * P:(g + 1) * P])
                x2 = wpool.tile([IN_F, P], BF16, tag="x2")
                nc.vector.tensor_tensor(out=x2[:], in0=xt[:], in1=xt[:],
                                        op=mybir.AluOpType.mult)
                x3 = wpool.tile([IN_F, P], BF16, tag="x3")
                nc.vector.tensor_tensor(out=x3[:], in0=x2[:], in1=xt[:],
                                        op=mybir.AluOpType.mult)
                ps = ppool.tile([P, HID_F], F32, tag="k1")
                nc.tensor.matmul(out=ps[:], lhsT=xt[:], rhs=c_w1[0][:],
                                 start=True, stop=False)
                nc.tensor.matmul(out=ps[:], lhsT=x2[:], rhs=c_w1[1][:],
                                 start=False, stop=False)
                nc.tensor.matmul(out=ps[:], lhsT=x3[:], rhs=c_w1[2][:],
                                 start=False, stop=True)
                hb = wpool.tile([P, HID_F], F32, tag="hb")
                nc.vector.tensor_tensor(out=hb[:], in0=ps[:], in1=c_b1[:],
                                        op=mybir.AluOpType.add)
                h1t = wpool.tile([P, HID_F], BF16, tag="h1t")
                nc.scalar.activation(out=h1t[:], in_=hb[:],
                                     func=mybir.ActivationFunctionType.Relu,
                                     scale=c_mask[:, g:g + 1])
                nc.sync.dma_start(out=h1_in[g * P:(g + 1) * P, :], in_=h1t[:])
                if dump:
                    nc.sync.dma_start(out=h1o[g * P:(g + 1) * P, :], in_=h1t[:])

            # ---------------- AllGather h1 ----------------
            nc.gpsimd.collective_compute(
                "AllGather", mybir.AluOpType.bypass,
                replica_groups=[list(range(K))],
                ins=[h1_in[:, :]], outs=[h1_tbl[:, :]],
            )

            # ---------------- phase 2: aggregate + KAN layer 2 ----------------
            for g in range(G):
                D = int(Dg[g])
                off = int(offs[g])
                gt = gpool.tile([P, Dmax * HID_F], BF16, tag="g1")
                for d in range(D):
                    nc.gpsimd.indirect_dma_start(
                        out=gt[:, d * HID_F:(d + 1) * HID_F],
                        out_offset=None,
                        in_=h1_tbl[:, :],
                        in_offset=bass.IndirectOffsetOnAxis(
                            ap=c_idx[:, off + d:off + d + 1], axis=0),
                    )
                s1 = _tree_reduce(nc, gt, D, HID_F)
                sn = wpool.tile([P, HID_F], F32, tag="sn")
                nc.vector.tensor_scalar_mul(sn[:], s1, c_dr[:, g:g + 1])
                if dump:
                    nc.sync.dma_start(out=sno[g * P:(g + 1) * P, :], in_=sn[:])
                pt = ppool.tile([P, P], F32, tag="tr")
                nc.tensor.transpose(out=pt[:], in_=sn[:], identity=c_id[:])
                hT = wpool.tile([HID_F, P], BF16, tag="hT")
                nc.scalar.copy(out=hT[:], in_=pt[:])
                q2 = wpool.tile([HID_F, P], BF16, tag="q2")
                nc.vector.tensor_tensor(out=q2[:], in0=hT[:], in1=hT[:],
                                        op=mybir.AluOpType.mult)
                q3 = wpool.tile([HID_F, P], BF16, tag="q3")
                nc.vector.tensor_tensor(out=q3[:], in0=q2[:], in1=hT[:],
                                        op=mybir.AluOpType.mult)
                ps2 = ppool.tile([P, OUT_F], F32, tag="k2")
                nc.tensor.matmul(out=ps2[:], lhsT=hT[:], rhs=c_w2[0][:],
                                 start=True, stop=False)
                nc.tensor.matmul(out=ps2[:], lhsT=q2[:], rhs=c_w2[1][:],
                                 start=False, stop=False)
                nc.tensor.matmul(out=ps2[:], lhsT=q3[:], rhs=c_w2[2][:],
                                 start=False, stop=True)
                hb2 = wpool.tile([P, OUT_F], F32, tag="hb2")
                nc.vector.tensor_tensor(out=hb2[:], in0=ps2[:], in1=c_b2[:],
                                        op=mybir.AluOpType.add)
                h2t = wpool.tile([P, OUT_F], BF16, tag="h2t")
                nc.scalar.activation(out=h2t[:], in_=hb2[:],
                                     func=mybir.ActivationFunctionType.Copy,
                                     scale=c_mask[:, g:g + 1])
                nc.sync.dma_start(out=h2_in[g * P:(g + 1) * P, :], in_=h2t[:])

            # ---------------- AllGather h2 ----------------
            nc.gpsimd.collective_compute(
                "AllGather", mybir.AluOpType.bypass,
                replica_groups=[list(range(K))],
                ins=[h2_in[:, :]], outs=[h2_tbl[:, :]],
            )

            # ---------------- phase 3: aggregate + log_softmax ----------------
            for g in range(G):
                D = int(Dg[g])
                off = int(offs[g])
                gt = gpool.tile([P, Dmax * OUT_F], BF16, tag="g2")
                for d in range(D):
                    nc.gpsimd.indirect_dma_start(
                        out=gt[:, d * OUT_F:(d + 1) * OUT_F],
                        out_offset=None,
                        in_=h2_tbl[:, :],
                        in_offset=bass.IndirectOffsetOnAxis(
                            ap=c_idx[:, off + d:off + d + 1], axis=0),
                    )
                s2 = _tree_reduce(nc, gt, D, OUT_F)
                tn = wpool.tile([P, OUT_F], F32, tag="tn")
                nc.vector.tensor_scalar_mul(tn[:], s2, c_dr[:, g:g + 1])
                mx = wpool.tile([P, 1], F32, tag="mx")
                nc.vector.tensor_reduce(out=mx[:], in_=tn[:],
                                        axis=mybir.AxisListType.X,
                                        op=mybir.AluOpType.max)
                nmx = wpool.tile([P, 1], F32, tag="nmx")
                nc.vector.tensor_scalar_mul(nmx[:], mx[:], -1.0)
                et = wpool.tile([P, OUT_F], F32, tag="et")
                se = wpool.tile([P, 1], F32, tag="se")
                nc.scalar.activation(out=et[:], in_=tn[:],
                                     func=mybir.ActivationFunctionType.Exp,
                                     bias=nmx[:, :1], scale=1.0,
                                     accum_out=se[:, :1])
                lse = wpool.tile([P, 1], F32, tag="lse")
                nc.scalar.activation(out=lse[:], in_=se[:],
                                     func=mybir.ActivationFunctionType.Ln)
                ot = wpool.tile([P, OUT_F], F32, tag="ot")
                nc.vector.tensor_scalar(ot[:], tn[:], nmx[:, :1], lse[:, :1],
                                        mybir.AluOpType.add,
                                        mybir.AluOpType.subtract)
                nc.sync.dma_start(out=y[g * P:(g + 1) * P, :], in_=ot[:])

    nc.compile()
    return nc


def kernel(x, edge_index, w1, b1, c1, w2, b2, c2):
    in_maps, meta = _host_prep(x, edge_index, w1, b1, c1, w2, b2, c2)
    nc = build_program(meta["Dg"], meta["offs"], meta["S"])
    res = run_bass_kernel_spmd(nc, in_maps, core_ids=list(range(K)))
    order = meta["order"]
    out = np.empty((N_NODES, OUT_F), dtype=np.float32)
    jr = np.arange(JREAL)
    for k in range(K):
        out[order[jr * K + k]] = res.results[k]["y"][:JREAL]
    return out
